# revision 1
# baseline (speedup 1.0000x reference)
"""Trainium2 Bass kernel for nn_MultiHeadSelfAttentionBlock — fp8 DoubleRow rewrite.

Strategy (data-parallel over batch, B=32 -> 4 per core on 8 cores):
  - All weight preprocessing (BN fold, transposes, tap-weight products, fp8
    quantization with power-of-2 prescales) done HOST-side in numpy; the
    kernel receives ready-to-use fp8 weight tensors. No on-chip setup phase.
  - All matmuls fp8e4m3 with DoubleRow perf mode (two K-half planes packed
    along the free dim of both lhsT and rhs -> 0.5 cyc/row).
  - BN applied on DVE writing xn8 [128, 6*1024] fp8 (chunk-major free, 6th
    chunk zeroed so K=640 pads to 3 DoubleRow pairs); gpsimd cannot produce
    fp8 on hardware (CoreSim permits it; the Q7 DSP emits garbage).
  - q proj: 24 DR matmuls -> qp psum [s-slice 128, oc 512]; PSUM->SBUF cast
    (scale 1/64) scatters into qbuf [128, 4096] (free = oc*8 + t, the torch
    .view-bug layout); SBUF->SBUF DMA reshuffles kd halves into
    qbuf2 [64, (kd_hi 2, head 8, l' 512)] for DR logits.
  - k/v conv: no im2col — 45 plain fp8 matmuls read a zero-padded
    [33 x 34]-per-chunk copy (xp8, persistent buffer, borders zeroed once)
    with strided tap views, all full 16x16 rects. k (cols 0:64) and
    v (cols 64:128) packed in one lhsT.
  - logits per (head, par, ptile): DR K=64 via kd-half pairs (kfDR built by
    4 tiny SBUF->SBUF DMAs, duplicated for both par base-partitions).
  - exp on ACT with scale 1/S_K and bias EXP_BIAS (folded shift; cancels in
    softmax), output fp8.
  - o-matmul: non-DR [65, 512] per (head, par, pt-accum) — DoubleRow
    requires output column position 0, so the baseline ones-column trick
    supplies the softmax denominator as psum row 64. Denominator rows are
    collected into a persistent dall tile (rows 0/32) and inverted with one
    reciprocal_approx_fast per head-pair.
  - normalize fused into PSUM->SBUF STT (x rec broadcast via SBUF->SBUF DMA),
    scatter-writing o_resh8 [128, (chunk 4, s 1024)] fp8 directly.
  - out proj: 20 DR matmuls; final residual fused: out = po * (ls/64) + x
    in one DVE scalar_tensor_tensor per half.
"""

from contextlib import ExitStack

import numpy as np

import concourse.bacc as bacc
import concourse.bass as bass
import concourse.tile as tile
from concourse import mybir
from concourse.masks import make_identity

BF16 = mybir.dt.bfloat16

F32 = mybir.dt.float32
F32R = mybir.dt.float32r
F8 = mybir.dt.float8e4
ALU = mybir.AluOpType
ACTF = mybir.ActivationFunctionType
DR = mybir.MatmulPerfMode.DoubleRow

B, C, H, W = 32, 640, 32, 32
NH, KD, VD = 8, 64, 64
S = H * W            # 1024
P = 256              # key/value positions (16x16)
EPS = 1e-3
N_CORES = 8
BPC = B // N_CORES   # 4 batch items per core
NCH = C // 128       # 5 channel chunks

# fp8 prescales (powers of two; descaled at PSUM->SBUF moves)
SW_Q = 64.0          # q_w
SW_K = 1024.0        # k tap weights (incl. the 1/8 logit scale)
SW_V = 256.0         # v tap weights
SW_O = 64.0          # out_w
S_K = 4.0            # kf8 = 4 * (k_true/8)  -> exp scale = 1/4
EXP_BIAS = 0.0       # actual |logit| max ~0.51 -> exp in [0.6, 1.7], safe for e4m3


def _r(ap):
    return ap.bitcast(F32R)


def _fap(base, free_off, dims):
    """AP with base's partition dim and explicit free dims [[step, count],...]."""
    return bass.AP(tensor=base.tensor, offset=base.offset + free_off,
                   ap=[base.ap[0]] + dims)


def build_nc(ls_scalar, nbatch=BPC, dbg=False):
    nc = bacc.Bacc(None, target_bir_lowering=False, debug=False)

    x4 = nc.dram_tensor("x", [nbatch, C, H, W], F32, kind="ExternalInput")
    qw8 = nc.dram_tensor("qw8", [128, 6 * 512], F8, kind="ExternalInput")
    wtap8 = nc.dram_tensor("wtap8", [128, 9 * 3 * 2 * 128], F8,
                           kind="ExternalInput")
    owt8 = nc.dram_tensor("owt8", [128, 2 * 2 * 640], F8, kind="ExternalInput")
    kconst = nc.dram_tensor("kconst", [64, 1], F32, kind="ExternalInput")
    vconst = nc.dram_tensor("vconst", [64, 1], F32, kind="ExternalInput")
    bn_sc = nc.dram_tensor("bn_sc", [128, NCH], F32, kind="ExternalInput")
    bn_sh = nc.dram_tensor("bn_sh", [128, NCH], F32, kind="ExternalInput")
    out4 = nc.dram_tensor("out", [nbatch, C, H, W], F32, kind="ExternalOutput")

    LS_IMM = float(ls_scalar) / SW_O

    with tile.TileContext(nc) as tc, ExitStack() as ctx:
        wp = ctx.enter_context(tc.tile_pool(name="wp", bufs=1))
        # PSUM pools: mmp(q/kv/po/transpose) 2 + lg 2 + op 4 = 8 banks
        mmp = ctx.enter_context(tc.tile_pool(name="mmp", bufs=2, space="PSUM"))
        lgp = ctx.enter_context(tc.tile_pool(name="lgp", bufs=2, space="PSUM"))
        opp = ctx.enter_context(tc.tile_pool(name="opp", bufs=4, space="PSUM"))
        # SBUF pools
        xin = ctx.enter_context(tc.tile_pool(name="xin", bufs=2))
        xnp = ctx.enter_context(tc.tile_pool(name="xnp", bufs=2))
        qbp = ctx.enter_context(tc.tile_pool(name="qbp", bufs=2))
        kvs = ctx.enter_context(tc.tile_pool(name="kvs", bufs=2))
        ep = ctx.enter_context(tc.tile_pool(name="ep", bufs=4))
        recp = ctx.enter_context(tc.tile_pool(name="recp", bufs=2))
        rbcp = ctx.enter_context(tc.tile_pool(name="rbcp", bufs=4))
        orp = ctx.enter_context(tc.tile_pool(name="orp", bufs=2))
        osb = ctx.enter_context(tc.tile_pool(name="osb", bufs=2))
        drp = ctx.enter_context(tc.tile_pool(name="drp", bufs=2, space="DRAM"))

        # ---- persistent SBUF weights (DMA'd once) ----
        ident = wp.tile([128, 128], F32, tag="ident", name="ident")
        make_identity(nc, ident[:])
        qw8_s = wp.tile([128, 6 * 512], F8, tag="qw8", name="qw8")
        nc.sync.dma_start(out=qw8_s[:], in_=qw8[:, :])
        wtap_s = wp.tile([128, 9 * 3 * 2 * 128], F8, tag="wtap", name="wtap")
        nc.sync.dma_start(out=wtap_s[:], in_=wtap8[:, :])
        owt_s = wp.tile([128, 2 * 2 * 640], F8, tag="owt", name="owt")
        nc.sync.dma_start(out=owt_s[:], in_=owt8[:, :])
        kc_s = wp.tile([64, 1], F32, tag="kc", name="kc")
        nc.sync.dma_start(out=kc_s[:], in_=kconst[:, :])
        vc_s = wp.tile([64, 1], F32, tag="vc", name="vc")
        nc.sync.dma_start(out=vc_s[:], in_=vconst[:, :])
        bnsc_s = wp.tile([128, NCH], F32, tag="bnsc", name="bnsc")
        nc.sync.dma_start(out=bnsc_s[:], in_=bn_sc[:, :])
        bnsh_s = wp.tile([128, NCH], F32, tag="bnsh", name="bnsh")
        nc.sync.dma_start(out=bnsh_s[:], in_=bn_sh[:, :])

        # persistent padded conv buffer + dall (single-buffer, setup-zeroed)
        xp8 = wp.tile([128, NCH * 33 * 34], F8, tag="xp8", name="xp8")
        for ch in range(NCH):
            nc.vector.memset(xp8[:, 33 * 34 * ch:33 * 34 * ch + 34], 0.0)
            nc.vector.memset(
                _fap(xp8[:], 33 * 34 * ch + 34, [[34, 32], [1, 1]]), 0.0)
        dall_t = wp.tile([33, 1024], F32, tag="dall", name="dall")
        nc.vector.memset(dall_t[:], 1.0)

        # conv taps on a zero-padded [33 x 34] plane per chunk (origin -1,-1):
        # tap (dy,dx) reads rows 2i+dy-1, cols 2j+dx-1 -> padded offset
        # dy*34+dx, strides (68, 2); every tap covers the full 16x16 rect.
        taps = [(dy, dx) for dy in range(3) for dx in range(3)]
        PADP = 33 * 34

        # ================= per batch item =================
        for b in range(nbatch):
            # ---- load x, BN -> xn8 fp8 [128, 6*1024] ----
            xts = []
            for ch in range(NCH):
                xt = xin.tile([128, 1024], F32, tag=f"xin{ch}", name=f"xin{ch}")
                nc.sync.dma_start(
                    out=xt[:],
                    in_=x4[b, 128 * ch:128 * (ch + 1), :, :].rearrange(
                        "c h w -> c (h w)"))
                xts.append(xt)
            xn8 = xnp.tile([128, 6 * 1024], F8, tag="xn8", name="xn8")
            nc.vector.memset(xn8[:, 5 * 1024:6 * 1024], 0.0)
            for ch in range(NCH):
                nc.vector.tensor_scalar(
                    out=xn8[:, 1024 * ch:1024 * (ch + 1)], in0=xts[ch][:],
                    scalar1=bnsc_s[:, ch:ch + 1], scalar2=bnsh_s[:, ch:ch + 1],
                    op0=ALU.mult, op1=ALU.add)
            for ch in range(NCH):
                nc.scalar.dma_start(
                    out=_fap(xp8[:], PADP * ch + 35, [[34, 32], [1, 32]]),
                    in_=xn8[:, 1024 * ch:1024 * (ch + 1)])

            # ---- q proj -> qbuf fp8 [128, 4096] (free = oc*8 + t) ----
            qbuf = qbp.tile([128, 4096], F8, tag="qbuf", name="qbuf")
            for t in range(8):
                qp = mmp.tile([128, 512], F32, tag="mm", name="qp")
                for k in range(3):
                    lhsT = _fap(xn8[:], 2048 * k + 128 * t, [[1024, 2], [1, 128]])
                    rhs = _fap(qw8_s[:], 1024 * k, [[512, 2], [1, 512]])
                    nc.tensor.matmul(qp[:], lhsT, rhs, start=(k == 0),
                                     stop=(k == 2), perf_mode=DR)
                dst = _fap(qbuf[:], t, [[8, 512]])
                if t % 2 == 0:
                    nc.scalar.activation(dst, qp[:], ACTF.Copy, scale=1.0 / SW_Q)
                else:
                    nc.vector.tensor_scalar_mul(dst, qp[:], 1.0 / SW_Q)

            # ---- k/v dwconv+BN+proj: direct strided taps, 27 DR matmuls ----
            kvp = mmp.tile([128, 256], F32, tag="mm", name="kvp")
            for ti, (dy, dx) in enumerate(taps):
                for ch in range(NCH):
                    lhsT = _fap(wtap_s[:],
                                768 * ti + 256 * (ch // 2) + 128 * (ch % 2),
                                [[1, 128]])
                    rhs = _fap(xp8[:], PADP * ch + 34 * dy + dx,
                               [[68, 16], [2, 16]])
                    nc.tensor.matmul(kvp[:], lhsT, rhs,
                                     start=(ti == 0 and ch == 0),
                                     stop=(ti == 8 and ch == NCH - 1))
            # kfdup [128, 256]: kf8 in rows 0:64 AND 64:128 (par base match)
            kfdup = kvs.tile([128, 256], F8, tag="kfdup", name="kfdup")
            nc.vector.tensor_scalar(out=kfdup[0:64, :], in0=kvp[0:64, :],
                                    scalar1=S_K / SW_K, scalar2=kc_s[:],
                                    op0=ALU.mult, op1=ALU.add)
            nc.vector.tensor_scalar(out=kfdup[64:128, :], in0=kvp[0:64, :],
                                    scalar1=S_K / SW_K, scalar2=kc_s[:],
                                    op0=ALU.mult, op1=ALU.add)
            vf = kvs.tile([64, 256], F32, tag="vf", name="vf")
            nc.vector.tensor_scalar(out=vf[:], in0=kvp[64:128, :],
                                    scalar1=1.0 / SW_V, scalar2=vc_s[:],
                                    op0=ALU.mult, op1=ALU.add)
            # vT8 [p 128, (pt 2, vd 64)] via PE transpose
            vT8 = kvs.tile([128, 130], F8, tag="vT8", name="vT8")
            nc.vector.memset(vT8[:, 64:65], 1.0)
            nc.vector.memset(vT8[:, 129:130], 1.0)
            for pt in range(2):
                tp = mmp.tile([128, 512], F32, tag="mm", name="tp")
                nc.tensor.transpose(tp[:128, 0:64],
                                    vf[:, 128 * pt:128 * (pt + 1)],
                                    ident[0:64, 0:64])
                nc.scalar.activation(vT8[:, 65 * pt:65 * pt + 64],
                                     tp[:128, 0:64], ACTF.Copy)

            # ---- attention ----
            # o-matmul is non-DR [65, 512] (DR requires out column position 0;
            # ones column in vT8 gives the softmax denominator as row 64)
            o_resh = orp.tile([128, 4096], F8, tag="oresh", name="oresh")
            for n2 in range(4):
                Es = {}
                for ni in range(2):
                    n = 2 * n2 + ni
                    for par in range(2):
                        E = ep.tile([128, 1024], F8, tag=f"E{par}",
                                    name=f"E{par}")
                        for pt in range(2):
                            lg = lgp.tile([128, 512], F32, tag="lg", name="lg")
                            lhsT = kfdup[64 * par:64 * (par + 1),
                                         128 * pt:128 * (pt + 1)]
                            rhs = qbuf[64 * par:64 * (par + 1),
                                       512 * n:512 * (n + 1)]
                            nc.tensor.matmul(lg[:], lhsT, rhs, start=True,
                                             stop=True)
                            nc.scalar.activation(
                                E[:, 512 * pt:512 * (pt + 1)], lg[:], ACTF.Exp,
                                bias=EXP_BIAS, scale=1.0 / S_K)
                        Es[(ni, par)] = E
                o_ps = {g: opp.tile([65, 512], F32, tag="op", name="op")
                        for g in Es}
                for pt in range(2):
                    vt_lhs = vT8[:, 65 * pt:65 * (pt + 1)]
                    for (ni, par), E in Es.items():
                        nc.tensor.matmul(
                            o_ps[(ni, par)][:], vt_lhs,
                            E[:, 512 * pt:512 * (pt + 1)],
                            start=(pt == 0), stop=(pt == 1))
                dall = dall_t
                for (ni, par), op_t in o_ps.items():
                    nc.vector.tensor_copy(
                        dall[32 * ni:32 * ni + 1, 512 * par:512 * (par + 1)],
                        op_t[64:65, :])
                rec = recp.tile([33, 1024], F32, tag="rec", name="rec")
                nc.vector.reciprocal_approx_fast(rec[:], dall[:])
                rec16 = recp.tile([33, 1024], BF16, tag="rec16", name="rec16")
                nc.vector.tensor_copy(rec16[:], rec[:])
                # bounce rows {0,32} through DRAM for partition-bcast
                dsc = drp.tile([2, 1024], BF16, tag="dsc", name="dsc")
                rec_rows = bass.AP(tensor=rec16.tensor, offset=rec16[:].offset,
                                   ap=[[rec16[:].ap[0][0] * 32, 2], [1, 1024]])
                nc.sync.dma_start(out=dsc[:], in_=rec_rows)
                for par in range(2):
                    rbc = rbcp.tile([128, 512], BF16, tag="rbc", name="rbc")
                    bsrc = bass.AP(tensor=dsc.tensor,
                                   offset=dsc[:].offset + 512 * par,
                                   ap=[[1024, 2], [0, 64], [1, 512]])
                    nc.sync.dma_start(out=rbc[:], in_=bsrc)
                    for ni in range(2):
                        out_ap = _fap(o_resh[64 * ni:64 * (ni + 1)],
                                      1024 * n2 + par, [[16, 64], [2, 8]])
                        nc.vector.scalar_tensor_tensor(
                            out=out_ap, in0=o_ps[(ni, par)][0:64, :],
                            scalar=1.0, in1=rbc[64 * ni:64 * (ni + 1), :],
                            op0=ALU.mult, op1=ALU.mult)

            if dbg and b == 0:
                # stash intermediates in out slots 1-3 (f32 casts)
                def dump(dst_b, dst_c0, src_ap, rows=128):
                    st = osb.tile([128, 1024], F32, tag="outsb", name="dmp")
                    if rows < 128:
                        nc.gpsimd.memset(st[:], 0.0)
                    n_free = 1
                    for s_, c_ in src_ap.ap[1:]:
                        n_free *= c_
                    nc.scalar.activation(st[0:rows, 0:n_free], src_ap,
                                         ACTF.Copy)
                    nc.sync.dma_start(
                        out=out4[dst_b, dst_c0:dst_c0 + 128, :, :].rearrange(
                            "c h w -> c (h w)"),
                        in_=st[:])
                for c4 in range(4):
                    dump(1, 128 * c4, qbuf[:, 1024 * c4:1024 * (c4 + 1)])
                dump(2, 0, xn8[:, 0:1024])
                dump(2, 128, _fap(xp8[:], 0, [[1, 1024]]))
                dump(2, 256, kf8[:, :], rows=64)
                dump(2, 384, vT8[:, :], rows=128)
                dump(3, 0, _fap(qbuf2[0:64], 0, [[1, 1024]]), rows=64)
                dump(3, 128, kfDR[:, :], rows=64)
                for c4 in range(3):
                    dump(3, 256 + 128 * c4,
                         o_resh[:, 1024 * c4:1024 * (c4 + 1)])
                dump(2, 512, o_resh[:, 3072:4096])

            # ---- out proj + layer scale + residual ----
            for ch in range(NCH):
                ot = osb.tile([128, 1024], F32, tag="outsb", name="outsb")
                for shalf in range(2):
                    po = mmp.tile([128, 512], F32, tag="mm", name="po")
                    for kp in range(2):
                        lhsT = _fap(owt_s[:], 1280 * kp + 128 * ch,
                                    [[640, 2], [1, 128]])
                        rhs = _fap(o_resh[:], 2048 * kp + 512 * shalf,
                                   [[1024, 2], [1, 512]])
                        nc.tensor.matmul(po[:], lhsT, rhs, start=(kp == 0),
                                         stop=(kp == 1), perf_mode=DR)
                    sl = slice(512 * shalf, 512 * (shalf + 1))
                    nc.vector.scalar_tensor_tensor(
                        out=ot[:, sl], in0=po[:], scalar=LS_IMM,
                        in1=xts[ch][:, sl], op0=ALU.mult, op1=ALU.add)
                nc.scalar.dma_start(
                    out=out4[b, 128 * ch:128 * (ch + 1), :, :].rearrange(
                        "c h w -> c (h w)"),
                    in_=ot[:])

    nc.finalize()
    return nc


def _prep_weights(inputs):
    f8 = mybir.dt.np(F8)
    g = {k: np.asarray(v, dtype=np.float32) for k, v in inputs.items()}

    def bnfold(p):
        sc = g[f"{p}_bn_gamma"] / np.sqrt(g[f"{p}_bn_var"] + EPS)
        sh = g[f"{p}_bn_beta"] - g[f"{p}_bn_mean"] * sc
        return sc, sh

    sc_in, sh_in = bnfold("in")
    sc_k, sh_k = bnfold("k")
    sc_v, sh_v = bnfold("v")

    # qw8 [c%128, (chunk 6, oc 512)] = q_w[oc, c] * SW_Q
    qw8 = np.zeros((128, 6, 512), np.float32)
    qwT = g["q_w"].T * SW_Q                     # [c, oc]
    qw8[:, :NCH, :] = qwT.reshape(NCH, 128, 512).transpose(1, 0, 2)
    # wtap8 [c%128, (tap 9, kpair 3, two 2, col 128)]; col 0:64 k, 64:128 v
    # tap order matches tapgeo: (1,1) first, then the remaining 8
    taps = [(dy, dx) for dy in range(3) for dx in range(3)]
    wtap = np.zeros((128, 9, 3, 2, 128), np.float32)
    kwT = g["k_w"].T * (sc_k[:, None] * (SW_K / 8.0))    # [c, kd]
    vwT = g["v_w"].T * (sc_v[:, None] * SW_V)            # [c, vd]
    dwk = g["k_dw_w"][:, 0]                              # [c, 3, 3]
    dwv = g["v_dw_w"][:, 0]
    for ti, (dy, dx) in enumerate(taps):
        for ch in range(NCH):
            cs = slice(128 * ch, 128 * (ch + 1))
            wtap[:, ti, ch // 2, ch % 2, 0:64] = kwT[cs] * dwk[cs, dy, dx][:, None]
            wtap[:, ti, ch // 2, ch % 2, 64:128] = vwT[cs] * dwv[cs, dy, dx][:, None]
    # owt8 [r, (kpair 2, two 2, c 640)] = out_w[c, nv]*SW_O, nv=(2kp+two)*128+r
    owt = np.zeros((128, 2, 2, 640), np.float32)
    owT = g["out_w"].T * SW_O                    # [nv, c]
    for kp in range(2):
        for two in range(2):
            nv0 = (2 * kp + two) * 128
            owt[:, kp, two, :] = owT[nv0:nv0 + 128]
    kconst = (g["k_w"] @ sh_k)[:, None] * (S_K / 8.0)
    vconst = (g["v_w"] @ sh_v)[:, None]
    bn_sc = sc_in.reshape(NCH, 128).T.copy()
    bn_sh = sh_in.reshape(NCH, 128).T.copy()
    return {
        "qw8": qw8.reshape(128, -1).astype(f8),
        "wtap8": wtap.reshape(128, -1).astype(f8),
        "owt8": owt.reshape(128, -1).astype(f8),
        "kconst": kconst.astype(np.float32),
        "vconst": vconst.astype(np.float32),
        "bn_sc": np.ascontiguousarray(bn_sc),
        "bn_sh": np.ascontiguousarray(bn_sh),
    }


_NC_CACHE = None


def kernel(**inputs):
    global _NC_CACHE
    from concourse.bass_utils import run_bass_kernel_spmd

    ls = np.asarray(inputs["ls_gamma"], dtype=np.float32)
    if _NC_CACHE is None:
        _NC_CACHE = build_nc(float(ls[0]))
    nc = _NC_CACHE

    x = np.ascontiguousarray(np.asarray(inputs["x"], dtype=np.float32))
    base = _prep_weights(inputs)
    in_maps = []
    for c in range(N_CORES):
        m = dict(base)
        m["x"] = x[c * BPC:(c + 1) * BPC]
        in_maps.append(m)

    res = run_bass_kernel_spmd(nc, in_maps, core_ids=list(range(N_CORES)))
    out = np.concatenate([res.results[c]["out"] for c in range(N_CORES)], axis=0)
    return out.astype(np.float32)



# revision 52
# speedup vs baseline: 3.3502x; 3.3502x over previous
"""Trainium2 Bass kernel for nn_MultiHeadSelfAttentionBlock (fp8 DoubleRow).

Data-parallel over batch (B=32 -> 4 per core on 8 cores). Weight prep
(BN fold, transposes, tap-weight products, fp8 quantization with
power-of-2 prescales) is host-side; the kernel gets ready fp8 weights.

Per batch item, software-pipelined (front/back emitted as thunks drained
between attention groups so ACT never starves while the lg ring limits
its run-ahead):
  - front: BN (DVE, per-partition scale/shift) -> xn8 fp8 [128, 6*1024];
    row-phase-separated conv planes xph via 10 SBUF->SBUF DMAs; q proj
    (24 DR matmuls + PSUM->SBUF casts split ACT/DVE into the torch
    .view-bug qbuf layout); k/v dwconv+BN+proj as 27 DR matmuls over the
    phase planes (stride-2 flat taps, 271-wide with ignored junk cols);
    v transposed via PE into two zero-padded DR lhsT variants (vtA/vtB).
  - attention (per head-pair n2, groups (ni, par)): logits DR-input
    matmuls [p 128, l 512] -> 2-bank lg tile; one exp per group on ACT
    (scale 1/S_K) -> E fp8 [128, (pt, l)]; o-matmul = one DR matmul per
    group accumulating ni=0/1 into a pair-packed [128, 512] psum tile;
    softmax denominators land partition-packed via 8 tiny [l' 128, 1]
    matmuls per group (lhsT = E l-slices, rhs = ones col); per n2: one
    reciprocal_approx_fast [128, 16], PE transpose into spare columns of
    the same psum bank, bf16 DRAM bounce + partition-broadcast DMA, and
    one normalize STT [128, 512] per par scatter-writing o_resh fp8.
  - back: out proj (20 DR matmuls) + residual STT out = po*(ls/64) + x
    on DVE, per-chunk output DMA on the Pool queue.
"""

from contextlib import ExitStack

import numpy as np

import concourse.bacc as bacc
import concourse.bass as bass
import concourse.tile as tile
from concourse import mybir
from concourse.masks import make_identity

BF16 = mybir.dt.bfloat16

F32 = mybir.dt.float32
F32R = mybir.dt.float32r
F8 = mybir.dt.float8e4
ALU = mybir.AluOpType
ACTF = mybir.ActivationFunctionType
DR = mybir.MatmulPerfMode.DoubleRow

B, C, H, W = 32, 640, 32, 32
NH, KD, VD = 8, 64, 64
S = H * W            # 1024
P = 256              # key/value positions (16x16)
EPS = 1e-3
N_CORES = 8
BPC = B // N_CORES   # 4 batch items per core
NCH = C // 128       # 5 channel chunks

# fp8 prescales (powers of two; descaled at PSUM->SBUF moves)
SW_Q = 64.0          # q_w
SW_K = 1024.0        # k tap weights (incl. the 1/8 logit scale)
SW_V = 256.0         # v tap weights
SW_O = 64.0          # out_w
S_K = 4.0            # kf8 = 4 * (k_true/8)  -> exp scale = 1/4
EXP_BIAS = 0.0       # actual |logit| max ~0.51 -> exp in [0.6, 1.7], safe for e4m3


def _r(ap):
    return ap.bitcast(F32R)


def _fap(base, free_off, dims):
    """AP with base's partition dim and explicit free dims [[step, count],...]."""
    return bass.AP(tensor=base.tensor, offset=base.offset + free_off,
                   ap=[base.ap[0]] + dims)


def build_nc(ls_scalar, nbatch=BPC, dbg=False):
    nc = bacc.Bacc(None, target_bir_lowering=False, debug=False)

    x4 = nc.dram_tensor("x", [nbatch, C, H, W], F32, kind="ExternalInput")
    qw8 = nc.dram_tensor("qw8", [128, 6 * 512], F8, kind="ExternalInput")
    wtap8 = nc.dram_tensor("wtap8", [128, 9 * 3 * 2 * 128], F8,
                           kind="ExternalInput")
    owt8 = nc.dram_tensor("owt8", [128, 2 * 2 * 640], F8, kind="ExternalInput")
    kconst = nc.dram_tensor("kconst", [64, 1], F32, kind="ExternalInput")
    vconst = nc.dram_tensor("vconst", [64, 1], F32, kind="ExternalInput")
    bn_sc = nc.dram_tensor("bn_sc", [128, NCH], F32, kind="ExternalInput")
    bn_sh = nc.dram_tensor("bn_sh", [128, NCH], F32, kind="ExternalInput")
    out4 = nc.dram_tensor("out", [nbatch, C, H, W], F32, kind="ExternalOutput")

    LS_IMM = float(ls_scalar) / SW_O

    with tile.TileContext(nc) as tc, ExitStack() as ctx:
        wp = ctx.enter_context(tc.tile_pool(name="wp", bufs=1))
        # PSUM banks: mmp 1 + lg 2x2 + op 2 + dall(+rec transpose) 1 = 8
        mmp = ctx.enter_context(tc.tile_pool(name="mmp", bufs=1, space="PSUM"))
        lgp = ctx.enter_context(tc.tile_pool(name="lgp", bufs=2, space="PSUM"))
        opp = ctx.enter_context(tc.tile_pool(name="opp", bufs=1, space="PSUM"))
        dalp = ctx.enter_context(tc.tile_pool(name="dalp", bufs=1,
                                              space="PSUM"))
        # SBUF pools
        xin = ctx.enter_context(tc.tile_pool(name="xin", bufs=3))
        xnp = ctx.enter_context(tc.tile_pool(name="xnp", bufs=3))
        qbp = ctx.enter_context(tc.tile_pool(name="qbp", bufs=3))
        kvs = ctx.enter_context(tc.tile_pool(name="kvs", bufs=3))
        ep = ctx.enter_context(tc.tile_pool(name="ep", bufs=6))
        recp = ctx.enter_context(tc.tile_pool(name="recp", bufs=3))
        rbcp = ctx.enter_context(tc.tile_pool(name="rbcp", bufs=6))
        orp = ctx.enter_context(tc.tile_pool(name="orp", bufs=2))
        osb = ctx.enter_context(tc.tile_pool(name="osb", bufs=2))
        drp = ctx.enter_context(tc.tile_pool(name="drp", bufs=4, space="DRAM"))

        # ---- persistent SBUF weights (DMA'd once) ----
        ident = wp.tile([128, 128], F32, tag="ident", name="ident")
        make_identity(nc, ident[:])
        qw8_s = wp.tile([128, 6 * 512], F8, tag="qw8", name="qw8")
        nc.scalar.dma_start(out=qw8_s[:], in_=qw8[:, :])
        wtap_s = wp.tile([128, 9 * 3 * 2 * 128], F8, tag="wtap", name="wtap")
        nc.gpsimd.dma_start(out=wtap_s[:], in_=wtap8[:, :])
        owt_s = wp.tile([128, 2 * 2 * 640], F8, tag="owt", name="owt")
        nc.scalar.dma_start(out=owt_s[:], in_=owt8[:, :])
        kc_s = wp.tile([64, 1], F32, tag="kc", name="kc")
        nc.scalar.dma_start(out=kc_s[:], in_=kconst[:, :])
        vc_s = wp.tile([64, 1], F32, tag="vc", name="vc")
        nc.scalar.dma_start(out=vc_s[:], in_=vconst[:, :])
        bnsc_s = wp.tile([128, NCH], F32, tag="bnsc", name="bnsc")
        nc.gpsimd.dma_start(out=bnsc_s[:], in_=bn_sc[:, :])
        bnsh_s = wp.tile([128, NCH], F32, tag="bnsh", name="bnsh")
        nc.gpsimd.dma_start(out=bnsh_s[:], in_=bn_sh[:, :])

        # persistent row-phase-separated conv buffer, setup-zeroed.
        # Per chunk: 2 planes [17 rows x 34 cols] (pr = h parity; row r2 holds
        # padded image row 2*r2+pr, data cols 1..32). 6 chunks (chunk 5 zero)
        # so the 45 conv planes pad to 27 DR pairs (wtap8's kpair=2,two=1
        # slot is zero host-side to match). Tap (dy,dx) = one stride-2 flat
        # run of 271 elements per plane (17 junk cols/row at j=16, ignored).
        PLN = 17 * 34        # 578, one phase plane
        CHG = 2 * PLN        # 1156, one chunk (both phases)
        xph = wp.tile([128, 6 * CHG], F8, tag="xph", name="xph")
        ones8 = wp.tile([128, 1], F8, tag="ones8", name="ones8")
        nc.vector.memset(ones8[:], 1.0)
        nc.gpsimd.memset(xph[:, 0:3 * CHG], 0.0)
        nc.gpsimd.memset(xph[:, 3 * CHG:6 * CHG], 0.0)
        taps = [(dy, dx) for dy in range(3) for dx in range(3)]


        # ============ software-pipelined batch loop ============
        # front/back phases are emitted as small thunks drained between
        # attention groups, so no engine sees a multi-us block of foreign
        # work while the lg ring (depth 2) caps ACT run-ahead.
        from collections import deque

        def load_x(b):
            xall = xin.tile([128, NCH * 1024], F32, tag="xall", name="xall")
            nc.sync.dma_start(
                out=xall[:],
                in_=bass.AP(tensor=x4, offset=b * C * S,
                            ap=[[1024, 128], [128 * 1024, NCH], [1, 1024]]))
            return xall

        def front_mk(b, xall):
            st = {"xall": xall}
            xts = [xall[:, 1024 * ch:1024 * (ch + 1)] for ch in range(NCH)]
            st["xts"] = xts
            xn8 = xnp.tile([128, 6 * 1024], F8, tag="xn8", name="xn8")
            qbuf = qbp.tile([128, 4096], F8, tag="qbuf", name="qbuf")
            st["qbuf"] = qbuf
            kvpt = mmp.tile([128, 512], F32, tag="mm", name="kvp")
            kfdup = kvs.tile([128, 256], F8, tag="kfdup", name="kfdup")
            st["kfdup"] = kfdup
            vf = kvs.tile([64, 256], F32, tag="vf", name="vf")
            vtA = kvs.tile([128, 256], F8, tag="vtA", name="vtA")
            vtB = kvs.tile([128, 256], F8, tag="vtB", name="vtB")
            st["vtA"], st["vtB"] = vtA, vtB
            thunks = []

            def bn_ch(ch):
                nc.vector.tensor_scalar(
                    out=xn8[:, 1024 * ch:1024 * (ch + 1)], in0=xts[ch][:],
                    scalar1=bnsc_s[:, ch:ch + 1], scalar2=bnsh_s[:, ch:ch + 1],
                    op0=ALU.mult, op1=ALU.add)
                for pr in range(2):
                    h0 = 1 - pr
                    r20 = 1 - pr
                    nc.gpsimd.dma_start(
                        out=_fap(xph[:], CHG * ch + PLN * pr + 34 * r20 + 1,
                                 [[34, 16], [1, 32]]),
                        in_=_fap(xn8[:], 1024 * ch + 32 * h0,
                                 [[64, 16], [1, 32]]))

            thunks.append(lambda: nc.gpsimd.memset(
                xn8[:, 5 * 1024:6 * 1024], 0.0))
            for ch in range(NCH):
                thunks.append(lambda ch=ch: bn_ch(ch))

            def q_t(t):
                qp = mmp.tile([128, 512], F32, tag="mm", name="qp")
                for k in range(3):
                    lhsT = _fap(xn8[:], 2048 * k + 128 * t,
                                [[1024, 2], [1, 128]])
                    rhs = _fap(qw8_s[:], 1024 * k, [[512, 2], [1, 512]])
                    nc.tensor.matmul(qp[:], lhsT, rhs, start=(k == 0),
                                     stop=(k == 2), perf_mode=DR)
                dst = _fap(qbuf[:], t, [[8, 512]])
                if t % 2 == 0:
                    nc.scalar.activation(dst, qp[:], ACTF.Copy,
                                         scale=1.0 / SW_Q)
                else:
                    nc.vector.tensor_scalar_mul(dst, qp[:], 1.0 / SW_Q)

            for t in range(8):
                thunks.append(lambda t=t: q_t(t))

            def conv_part(tis):
                for ti in tis:
                    dy, dx = taps[ti]
                    pr = dy % 2
                    toff = PLN * pr + 34 * (dy // 2) + dx
                    for kp in range(3):
                        lhsT = _fap(wtap_s[:], 768 * ti + 256 * kp,
                                    [[128, 2], [1, 128]])
                        rhs = _fap(xph[:], CHG * 2 * kp + toff,
                                   [[CHG, 2], [2, 271]])
                        nc.tensor.matmul(kvpt[:, 0:271], lhsT, rhs,
                                         start=(ti == 0 and kp == 0),
                                         stop=(ti == 8 and kp == 2),
                                         perf_mode=DR)

            thunks.append(lambda: conv_part([0, 1, 2, 3]))
            thunks.append(lambda: conv_part([4, 5, 6, 7, 8]))

            def kv_post():
                gcols = [[17, 16], [1, 16]]
                nc.vector.tensor_scalar(out=kfdup[0:64, :],
                                        in0=_fap(kvpt[0:64], 0, gcols),
                                        scalar1=S_K / SW_K, scalar2=kc_s[:],
                                        op0=ALU.mult, op1=ALU.add)
                nc.vector.tensor_scalar(out=kfdup[64:128, :],
                                        in0=_fap(kvpt[0:64], 0, gcols),
                                        scalar1=S_K / SW_K, scalar2=kc_s[:],
                                        op0=ALU.mult, op1=ALU.add)
                nc.vector.tensor_scalar(out=vf[:],
                                        in0=_fap(kvpt[64:128], 0, gcols),
                                        scalar1=1.0 / SW_V, scalar2=vc_s[:],
                                        op0=ALU.mult, op1=ALU.add)
                nc.vector.memset(vtA[:, 64:128], 0.0)
                nc.vector.memset(vtA[:, 192:256], 0.0)
                nc.vector.memset(vtB[:, 0:64], 0.0)
                nc.vector.memset(vtB[:, 128:192], 0.0)

            def vt_mk(pt):
                tp = mmp.tile([128, 512], F32, tag="mm", name="tp")
                nc.tensor.transpose(tp[:128, 0:64],
                                    vf[:, 128 * pt:128 * (pt + 1)],
                                    ident[0:64, 0:64])
                nc.scalar.activation(vtA[:, 128 * pt:128 * pt + 64],
                                     tp[:128, 0:64], ACTF.Copy)
                nc.scalar.activation(vtB[:, 128 * pt + 64:128 * pt + 128],
                                     tp[:128, 0:64], ACTF.Copy)

            thunks.append(kv_post)
            thunks.append(lambda: vt_mk(0))
            thunks.append(lambda: vt_mk(1))
            return st, thunks

        def attn(st, fill):
            qbuf, kfdup = st["qbuf"], st["kfdup"]
            vtA, vtB = st["vtA"], st["vtB"]
            st["o_resh"] = orp.tile([128, 4096], F8, tag="oresh",
                                    name="oresh")
            o_resh = st["o_resh"]

            def filler():
                if fill:
                    fill.popleft()()

            for n2 in range(4):
                opt = opp.tile([128, 1024], F32, tag="op", name="op")
                o_ps = {par: opt[:, 512 * par:512 * (par + 1)]
                        for par in range(2)}
                dallT = dalp.tile([128, 160], F32, tag="dallT", name="dallT")
                for ni in range(2):
                    n = 2 * n2 + ni
                    for par in range(2):
                        E = ep.tile([128, 1024], F8, tag=f"E{par}",
                                    name=f"E{par}")
                        lg = lgp.tile([128, 1024], F32, tag="lg", name="lg")
                        for pt in range(2):
                            lhsT = kfdup[64 * par:64 * (par + 1),
                                         128 * pt:128 * (pt + 1)]
                            rhs = qbuf[64 * par:64 * (par + 1),
                                       512 * n:512 * (n + 1)]
                            nc.tensor.matmul(lg[:, 512 * pt:512 * (pt + 1)],
                                             lhsT, rhs, start=True, stop=True)
                        nc.scalar.activation(E[:], lg[:], ACTF.Exp,
                                             bias=EXP_BIAS, scale=1.0 / S_K)
                        vt = vtA if ni == 0 else vtB
                        nc.tensor.matmul(
                            o_ps[par],
                            _fap(vt[:], 0, [[128, 2], [1, 128]]),
                            _fap(E[:], 0, [[512, 2], [1, 512]]),
                            start=(ni == 0), stop=(ni == 1), perf_mode=DR)
                        for lt in range(4):
                            c = 8 * par + 4 * ni + lt
                            for pt in range(2):
                                nc.tensor.matmul(
                                    dallT[:, c:c + 1],
                                    E[:, 512 * pt + 128 * lt:
                                        512 * pt + 128 * (lt + 1)],
                                    ones8[:, 0:1],
                                    start=(pt == 0), stop=(pt == 1))
                        filler()
                # denominator chain: reciprocal, PE transpose (same bank),
                # DRAM bounce, partition-bcast, normalize STT
                rec_sb = recp.tile([128, 16], F32, tag="rec", name="rec")
                nc.vector.reciprocal_approx_fast(rec_sb[:], dallT[:, 0:16])
                nc.tensor.transpose(dallT[0:16, 16:144], rec_sb[:],
                                    ident[:, :])
                recT = recp.tile([16, 128], BF16, tag="recT", name="recT")
                nc.vector.tensor_copy(recT[:], dallT[0:16, 16:144])
                dscD = drp.tile([16, 128], BF16, tag="dscD", name="dscD")
                nc.gpsimd.dma_start(out=dscD[:], in_=recT[:])
                filler()
                for par in range(2):
                    rbc = rbcp.tile([128, 512], BF16, tag="rbc", name="rbc")
                    bsrc = bass.AP(tensor=dscD.tensor,
                                   offset=dscD[:].offset + 8 * par * 128,
                                   ap=[[4 * 128, 2], [0, 64], [128, 4],
                                       [1, 128]])
                    nc.gpsimd.dma_start(out=rbc[:], in_=bsrc)
                    out_ap = _fap(o_resh[:], 1024 * n2 + par,
                                  [[16, 64], [2, 8]])
                    nc.vector.scalar_tensor_tensor(
                        out=out_ap, in0=o_ps[par],
                        scalar=1.0, in1=rbc[:],
                        op0=ALU.mult, op1=ALU.mult)
                filler()

        def back_mk(b, st):
            o_resh, xts = st["o_resh"], st["xts"]
            ot_all = osb.tile([128, NCH * 1024], F32, tag="outsb",
                              name="outsb")
            thunks = []

            def ch_step(ch):
                ot = ot_all[:, 1024 * ch:1024 * (ch + 1)]
                for shalf in range(2):
                    po = mmp.tile([128, 512], F32, tag="mm", name="po")
                    for kp in range(2):
                        lhsT = _fap(owt_s[:], 1280 * kp + 128 * ch,
                                    [[640, 2], [1, 128]])
                        rhs = _fap(o_resh[:], 2048 * kp + 512 * shalf,
                                   [[1024, 2], [1, 512]])
                        nc.tensor.matmul(po[:], lhsT, rhs, start=(kp == 0),
                                         stop=(kp == 1), perf_mode=DR)
                    sl = slice(512 * shalf, 512 * (shalf + 1))
                    nc.vector.scalar_tensor_tensor(
                        out=ot[:, sl], in0=po[:], scalar=LS_IMM,
                        in1=xts[ch][:, sl], op0=ALU.mult, op1=ALU.add)
                nc.sync.dma_start(
                    out=bass.AP(tensor=out4, offset=b * C * S + 131072 * ch,
                                ap=[[1024, 128], [1, 1024]]),
                    in_=ot[:, :])

            for ch in range(NCH):
                thunks.append(lambda ch=ch: ch_step(ch))
            return thunks

        xalls = {0: load_x(0)}
        sts = {}
        st0, th0 = front_mk(0, xalls[0])
        sts[0] = st0
        for t in th0:
            t()
        if nbatch > 1:
            xalls[1] = load_x(1)
        for b in range(nbatch):
            fill = deque()
            if b - 1 >= 0:
                fill.extend(back_mk(b - 1, sts.pop(b - 1)))
            if b + 1 < nbatch:
                if b + 2 < nbatch:
                    fill.append(lambda b=b: xalls.__setitem__(
                        b + 2, load_x(b + 2)))
                st_next, th_next = front_mk(b + 1, xalls[b + 1])
                sts[b + 1] = st_next
                fill.extend(th_next)
            attn(sts[b], fill)
            while fill:
                fill.popleft()()
        for t in back_mk(nbatch - 1, sts.pop(nbatch - 1)):
            t()

    nc.finalize()
    return nc


def _prep_weights(inputs):
    f8 = mybir.dt.np(F8)
    g = {k: np.asarray(v, dtype=np.float32) for k, v in inputs.items()}

    def bnfold(p):
        sc = g[f"{p}_bn_gamma"] / np.sqrt(g[f"{p}_bn_var"] + EPS)
        sh = g[f"{p}_bn_beta"] - g[f"{p}_bn_mean"] * sc
        return sc, sh

    sc_in, sh_in = bnfold("in")
    sc_k, sh_k = bnfold("k")
    sc_v, sh_v = bnfold("v")

    # qw8 [c%128, (chunk 6, oc 512)] = q_w[oc, c] * SW_Q
    qw8 = np.zeros((128, 6, 512), np.float32)
    qwT = g["q_w"].T * SW_Q                     # [c, oc]
    qw8[:, :NCH, :] = qwT.reshape(NCH, 128, 512).transpose(1, 0, 2)
    # wtap8 [c%128, (tap 9, kpair 3, two 2, col 128)]; col 0:64 k, 64:128 v
    # tap order matches tapgeo: (1,1) first, then the remaining 8
    taps = [(dy, dx) for dy in range(3) for dx in range(3)]
    wtap = np.zeros((128, 9, 3, 2, 128), np.float32)
    kwT = g["k_w"].T * (sc_k[:, None] * (SW_K / 8.0))    # [c, kd]
    vwT = g["v_w"].T * (sc_v[:, None] * SW_V)            # [c, vd]
    dwk = g["k_dw_w"][:, 0]                              # [c, 3, 3]
    dwv = g["v_dw_w"][:, 0]
    for ti, (dy, dx) in enumerate(taps):
        for ch in range(NCH):
            cs = slice(128 * ch, 128 * (ch + 1))
            wtap[:, ti, ch // 2, ch % 2, 0:64] = kwT[cs] * dwk[cs, dy, dx][:, None]
            wtap[:, ti, ch // 2, ch % 2, 64:128] = vwT[cs] * dwv[cs, dy, dx][:, None]
    # owt8 [r, (kpair 2, two 2, c 640)] = out_w[c, nv]*SW_O, nv=(2kp+two)*128+r
    owt = np.zeros((128, 2, 2, 640), np.float32)
    owT = g["out_w"].T * SW_O                    # [nv, c]
    for kp in range(2):
        for two in range(2):
            nv0 = (2 * kp + two) * 128
            owt[:, kp, two, :] = owT[nv0:nv0 + 128]
    kconst = (g["k_w"] @ sh_k)[:, None] * (S_K / 8.0)
    vconst = (g["v_w"] @ sh_v)[:, None]
    bn_sc = sc_in.reshape(NCH, 128).T.copy()
    bn_sh = sh_in.reshape(NCH, 128).T.copy()
    return {
        "qw8": qw8.reshape(128, -1).astype(f8),
        "wtap8": wtap.reshape(128, -1).astype(f8),
        "owt8": owt.reshape(128, -1).astype(f8),
        "kconst": kconst.astype(np.float32),
        "vconst": vconst.astype(np.float32),
        "bn_sc": np.ascontiguousarray(bn_sc),
        "bn_sh": np.ascontiguousarray(bn_sh),
    }


_NC_CACHE = None


def kernel(**inputs):
    global _NC_CACHE
    from concourse.bass_utils import run_bass_kernel_spmd

    ls = np.asarray(inputs["ls_gamma"], dtype=np.float32)
    if _NC_CACHE is None:
        _NC_CACHE = build_nc(float(ls[0]))
    nc = _NC_CACHE

    x = np.ascontiguousarray(np.asarray(inputs["x"], dtype=np.float32))
    base = _prep_weights(inputs)
    in_maps = []
    for c in range(N_CORES):
        m = dict(base)
        m["x"] = x[c * BPC:(c + 1) * BPC]
        in_maps.append(m)

    res = run_bass_kernel_spmd(nc, in_maps, core_ids=list(range(N_CORES)))
    out = np.concatenate([res.results[c]["out"] for c in range(N_CORES)], axis=0)
    return out.astype(np.float32)



# revision 63
# speedup vs baseline: 3.5071x; 1.0468x over previous
"""Trainium2 Bass kernel for nn_MultiHeadSelfAttentionBlock (fp8 DoubleRow).

Data-parallel over batch (B=32 -> 4 per core on 8 cores). Weight prep
(BN fold, transposes, tap-weight products, fp8 quantization with
power-of-2 prescales) is host-side; the kernel gets ready fp8 weights.

Per batch item, software-pipelined (front/back emitted as thunks drained
between attention groups so ACT never starves while the lg ring limits
its run-ahead):
  - front: BN (DVE, per-partition scale/shift) -> xn8 fp8 [128, 6*1024];
    row-phase-separated conv planes xph via 10 SBUF->SBUF DMAs; q proj
    (24 DR matmuls + DVE PSUM->SBUF casts into the torch .view-bug qbuf
    layout; ACT carries nothing but the exps so its stream never
    dilutes); k/v dwconv+BN+proj as 27 DR matmuls over the phase planes
    (stride-2 flat taps, 271-wide with ignored junk cols); kfdup row
    duplication via SBUF->SBUF DMA; v transposed via PE into two
    zero-padded DR lhsT variants (vtA/vtB).
  - attention (per head-pair n2, groups (ni, par)): logits DR-input
    matmuls [p 128, l 512] -> 2-bank lg tile; one exp per group on ACT
    (scale 1/S_K) -> E fp8 [128, (pt, l)]; o-matmul = one DR matmul per
    group accumulating ni=0/1 into a pair-packed [128, 512] psum tile;
    softmax denominators land partition-packed via 8 tiny [l' 128, 1]
    matmuls per group (lhsT = E l-slices, rhs = ones col); per n2: one
    reciprocal_approx_fast [128, 16], PE transpose into spare columns of
    the same psum bank, bf16 DRAM bounce + partition-broadcast DMA, and
    one normalize STT [128, 512] per par scatter-writing o_resh fp8.
  - back: out proj (20 DR matmuls) + residual STT out = po*(ls/64) + x
    on DVE, per-chunk output DMA on the SP queue (loads prefetched two
    batches ahead; queue placement chosen empirically since each DMA
    queue serializes its transfers).
"""

from contextlib import ExitStack

import numpy as np

import concourse.bacc as bacc
import concourse.bass as bass
import concourse.tile as tile
from concourse import mybir
from concourse.masks import make_identity

BF16 = mybir.dt.bfloat16

F32 = mybir.dt.float32
F32R = mybir.dt.float32r
F8 = mybir.dt.float8e4
ALU = mybir.AluOpType
ACTF = mybir.ActivationFunctionType
DR = mybir.MatmulPerfMode.DoubleRow

B, C, H, W = 32, 640, 32, 32
NH, KD, VD = 8, 64, 64
S = H * W            # 1024
P = 256              # key/value positions (16x16)
EPS = 1e-3
N_CORES = 8
BPC = B // N_CORES   # 4 batch items per core
NCH = C // 128       # 5 channel chunks

# fp8 prescales (powers of two; descaled at PSUM->SBUF moves)
SW_Q = 64.0          # q_w
SW_K = 1024.0        # k tap weights (incl. the 1/8 logit scale)
SW_V = 256.0         # v tap weights
SW_O = 64.0          # out_w
S_K = 4.0            # kf8 = 4 * (k_true/8)  -> exp scale = 1/4
EXP_BIAS = 0.0       # actual |logit| max ~0.51 -> exp in [0.6, 1.7], safe for e4m3


def _r(ap):
    return ap.bitcast(F32R)


def _fap(base, free_off, dims):
    """AP with base's partition dim and explicit free dims [[step, count],...]."""
    return bass.AP(tensor=base.tensor, offset=base.offset + free_off,
                   ap=[base.ap[0]] + dims)


def build_nc(ls_scalar, nbatch=BPC, dbg=False):
    nc = bacc.Bacc(None, target_bir_lowering=False, debug=False)

    x4 = nc.dram_tensor("x", [nbatch, C, H, W], F32, kind="ExternalInput")
    qw8 = nc.dram_tensor("qw8", [128, 6 * 512], F8, kind="ExternalInput")
    wtap8 = nc.dram_tensor("wtap8", [128, 9 * 3 * 2 * 128], F8,
                           kind="ExternalInput")
    owt8 = nc.dram_tensor("owt8", [128, 2 * 2 * 640], F8, kind="ExternalInput")
    kconst = nc.dram_tensor("kconst", [64, 1], F32, kind="ExternalInput")
    vconst = nc.dram_tensor("vconst", [64, 1], F32, kind="ExternalInput")
    bn_sc = nc.dram_tensor("bn_sc", [128, NCH], F32, kind="ExternalInput")
    bn_sh = nc.dram_tensor("bn_sh", [128, NCH], F32, kind="ExternalInput")
    out4 = nc.dram_tensor("out", [nbatch, C, H, W], F32, kind="ExternalOutput")

    LS_IMM = float(ls_scalar) / SW_O

    with tile.TileContext(nc) as tc, ExitStack() as ctx:
        wp = ctx.enter_context(tc.tile_pool(name="wp", bufs=1))
        # PSUM banks: mmp 1 + lg 2x2 + op 2 + dall(+rec transpose) 1 = 8
        mmp = ctx.enter_context(tc.tile_pool(name="mmp", bufs=1, space="PSUM"))
        lgp = ctx.enter_context(tc.tile_pool(name="lgp", bufs=2, space="PSUM"))
        opp = ctx.enter_context(tc.tile_pool(name="opp", bufs=1, space="PSUM"))
        dalp = ctx.enter_context(tc.tile_pool(name="dalp", bufs=1,
                                              space="PSUM"))
        # SBUF pools
        xin = ctx.enter_context(tc.tile_pool(name="xin", bufs=3))
        xnp = ctx.enter_context(tc.tile_pool(name="xnp", bufs=3))
        qbp = ctx.enter_context(tc.tile_pool(name="qbp", bufs=3))
        kvs = ctx.enter_context(tc.tile_pool(name="kvs", bufs=3))
        ep = ctx.enter_context(tc.tile_pool(name="ep", bufs=6))
        recp = ctx.enter_context(tc.tile_pool(name="recp", bufs=3))
        rbcp = ctx.enter_context(tc.tile_pool(name="rbcp", bufs=6))
        orp = ctx.enter_context(tc.tile_pool(name="orp", bufs=3))
        osb = ctx.enter_context(tc.tile_pool(name="osb", bufs=3))
        drp = ctx.enter_context(tc.tile_pool(name="drp", bufs=4, space="DRAM"))

        # ---- persistent SBUF weights (DMA'd once) ----
        ident = wp.tile([128, 128], F32, tag="ident", name="ident")
        make_identity(nc, ident[:])
        qw8_s = wp.tile([128, 6 * 512], F8, tag="qw8", name="qw8")
        nc.scalar.dma_start(out=qw8_s[:], in_=qw8[:, :])
        wtap_s = wp.tile([128, 9 * 3 * 2 * 128], F8, tag="wtap", name="wtap")
        nc.gpsimd.dma_start(out=wtap_s[:], in_=wtap8[:, :])
        owt_s = wp.tile([128, 2 * 2 * 640], F8, tag="owt", name="owt")
        nc.scalar.dma_start(out=owt_s[:], in_=owt8[:, :])
        kc_s = wp.tile([64, 1], F32, tag="kc", name="kc")
        nc.scalar.dma_start(out=kc_s[:], in_=kconst[:, :])
        vc_s = wp.tile([64, 1], F32, tag="vc", name="vc")
        nc.scalar.dma_start(out=vc_s[:], in_=vconst[:, :])
        bnsc_s = wp.tile([128, NCH], F32, tag="bnsc", name="bnsc")
        nc.gpsimd.dma_start(out=bnsc_s[:], in_=bn_sc[:, :])
        bnsh_s = wp.tile([128, NCH], F32, tag="bnsh", name="bnsh")
        nc.gpsimd.dma_start(out=bnsh_s[:], in_=bn_sh[:, :])

        # persistent row-phase-separated conv buffer, setup-zeroed.
        # Per chunk: 2 planes [17 rows x 34 cols] (pr = h parity; row r2 holds
        # padded image row 2*r2+pr, data cols 1..32). 6 chunks (chunk 5 zero)
        # so the 45 conv planes pad to 27 DR pairs (wtap8's kpair=2,two=1
        # slot is zero host-side to match). Tap (dy,dx) = one stride-2 flat
        # run of 271 elements per plane (17 junk cols/row at j=16, ignored).
        PLN = 17 * 34        # 578, one phase plane
        CHG = 2 * PLN        # 1156, one chunk (both phases)
        xph = wp.tile([128, 6 * CHG], F8, tag="xph", name="xph")
        ones8 = wp.tile([128, 1], F8, tag="ones8", name="ones8")
        nc.vector.memset(ones8[:], 1.0)
        nc.gpsimd.memset(xph[:, 0:3 * CHG], 0.0)
        nc.gpsimd.memset(xph[:, 3 * CHG:6 * CHG], 0.0)
        taps = [(dy, dx) for dy in range(3) for dx in range(3)]


        # ============ software-pipelined batch loop ============
        # front/back phases are emitted as small thunks drained between
        # attention groups, so no engine sees a multi-us block of foreign
        # work while the lg ring (depth 2) caps ACT run-ahead.
        from collections import deque

        def load_x(b, split=False):
            xall = xin.tile([128, NCH * 1024], F32, tag="xall", name="xall")
            if split:
                # cold-start: halve the load latency across two idle queues
                nc.sync.dma_start(
                    out=xall[:, 0:3 * 1024],
                    in_=bass.AP(tensor=x4, offset=b * C * S,
                                ap=[[1024, 128], [128 * 1024, 3], [1, 1024]]))
                nc.gpsimd.dma_start(
                    out=xall[:, 3 * 1024:5 * 1024],
                    in_=bass.AP(tensor=x4,
                                offset=b * C * S + 3 * 128 * 1024,
                                ap=[[1024, 128], [128 * 1024, 2], [1, 1024]]))
                return xall
            nc.sync.dma_start(
                out=xall[:],
                in_=bass.AP(tensor=x4, offset=b * C * S,
                            ap=[[1024, 128], [128 * 1024, NCH], [1, 1024]]))
            return xall

        def front_mk(b, xall):
            st = {"xall": xall}
            xts = [xall[:, 1024 * ch:1024 * (ch + 1)] for ch in range(NCH)]
            st["xts"] = xts
            xn8 = xnp.tile([128, 6 * 1024], F8, tag="xn8", name="xn8")
            qbuf = qbp.tile([128, 4096], F8, tag="qbuf", name="qbuf")
            st["qbuf"] = qbuf
            kvpt = mmp.tile([128, 512], F32, tag="mm", name="kvp")
            kfdup = kvs.tile([128, 256], F8, tag="kfdup", name="kfdup")
            st["kfdup"] = kfdup
            vf = kvs.tile([64, 256], F32, tag="vf", name="vf")
            vtA = kvs.tile([128, 256], F8, tag="vtA", name="vtA")
            vtB = kvs.tile([128, 256], F8, tag="vtB", name="vtB")
            st["vtA"], st["vtB"] = vtA, vtB
            thunks = []

            def bn_ch(ch):
                nc.vector.tensor_scalar(
                    out=xn8[:, 1024 * ch:1024 * (ch + 1)], in0=xts[ch][:],
                    scalar1=bnsc_s[:, ch:ch + 1], scalar2=bnsh_s[:, ch:ch + 1],
                    op0=ALU.mult, op1=ALU.add)
                for pr in range(2):
                    h0 = 1 - pr
                    r20 = 1 - pr
                    nc.gpsimd.dma_start(
                        out=_fap(xph[:], CHG * ch + PLN * pr + 34 * r20 + 1,
                                 [[34, 16], [1, 32]]),
                        in_=_fap(xn8[:], 1024 * ch + 32 * h0,
                                 [[64, 16], [1, 32]]))

            thunks.append(lambda: nc.gpsimd.memset(
                xn8[:, 5 * 1024:6 * 1024], 0.0))
            for ch in range(NCH):
                thunks.append(lambda ch=ch: bn_ch(ch))

            def q_t(t):
                qp = mmp.tile([128, 512], F32, tag="mm", name="qp")
                for k in range(3):
                    lhsT = _fap(xn8[:], 2048 * k + 128 * t,
                                [[1024, 2], [1, 128]])
                    rhs = _fap(qw8_s[:], 1024 * k, [[512, 2], [1, 512]])
                    nc.tensor.matmul(qp[:], lhsT, rhs, start=(k == 0),
                                     stop=(k == 2), perf_mode=DR)
                dst = _fap(qbuf[:], t, [[8, 512]])
                nc.vector.tensor_scalar_mul(dst, qp[:], 1.0 / SW_Q)

            for t in range(8):
                thunks.append(lambda t=t: q_t(t))

            def conv_part(tis):
                for ti in tis:
                    dy, dx = taps[ti]
                    pr = dy % 2
                    toff = PLN * pr + 34 * (dy // 2) + dx
                    for kp in range(3):
                        lhsT = _fap(wtap_s[:], 768 * ti + 256 * kp,
                                    [[128, 2], [1, 128]])
                        rhs = _fap(xph[:], CHG * 2 * kp + toff,
                                   [[CHG, 2], [2, 271]])
                        nc.tensor.matmul(kvpt[:, 0:271], lhsT, rhs,
                                         start=(ti == 0 and kp == 0),
                                         stop=(ti == 8 and kp == 2),
                                         perf_mode=DR)

            thunks.append(lambda: conv_part([0, 1, 2, 3]))
            thunks.append(lambda: conv_part([4, 5, 6, 7, 8]))

            def kv_post():
                gcols = [[17, 16], [1, 16]]
                nc.vector.tensor_scalar(out=kfdup[0:64, :],
                                        in0=_fap(kvpt[0:64], 0, gcols),
                                        scalar1=S_K / SW_K, scalar2=kc_s[:],
                                        op0=ALU.mult, op1=ALU.add)
                nc.sync.dma_start(out=kfdup[64:128, :], in_=kfdup[0:64, :])
                nc.vector.tensor_scalar(out=vf[:],
                                        in0=_fap(kvpt[64:128], 0, gcols),
                                        scalar1=1.0 / SW_V, scalar2=vc_s[:],
                                        op0=ALU.mult, op1=ALU.add)
                nc.gpsimd.memset(vtA[:, 64:128], 0.0)
                nc.gpsimd.memset(vtA[:, 192:256], 0.0)
                nc.gpsimd.memset(vtB[:, 0:64], 0.0)
                nc.gpsimd.memset(vtB[:, 128:192], 0.0)

            def vt_mk(pt):
                tp = mmp.tile([128, 512], F32, tag="mm", name="tp")
                nc.tensor.transpose(tp[:128, 0:64],
                                    vf[:, 128 * pt:128 * (pt + 1)],
                                    ident[0:64, 0:64])
                nc.vector.tensor_copy(vtA[:, 128 * pt:128 * pt + 64],
                                       tp[:128, 0:64])
                nc.vector.tensor_copy(vtB[:, 128 * pt + 64:128 * pt + 128],
                                      tp[:128, 0:64])

            thunks.append(kv_post)
            thunks.append(lambda: vt_mk(0))
            thunks.append(lambda: vt_mk(1))
            return st, thunks

        def attn(st, fill):
            qbuf, kfdup = st["qbuf"], st["kfdup"]
            vtA, vtB = st["vtA"], st["vtB"]
            st["o_resh"] = orp.tile([128, 4096], F8, tag="oresh",
                                    name="oresh")
            o_resh = st["o_resh"]
            def filler():
                if fill:
                    fill.popleft()()

            for n2 in range(4):
                opt = opp.tile([128, 1024], F32, tag="op", name="op")
                o_ps = {par: opt[:, 512 * par:512 * (par + 1)]
                        for par in range(2)}
                dallT = dalp.tile([128, 160], F32, tag="dallT", name="dallT")
                for ni in range(2):
                    n = 2 * n2 + ni
                    for par in range(2):
                        E = ep.tile([128, 1024], F8, tag=f"E{par}",
                                    name=f"E{par}")
                        lg = lgp.tile([128, 1024], F32, tag="lg", name="lg")
                        for pt in range(2):
                            lhsT = kfdup[64 * par:64 * (par + 1),
                                         128 * pt:128 * (pt + 1)]
                            rhs = qbuf[64 * par:64 * (par + 1),
                                       512 * n:512 * (n + 1)]
                            nc.tensor.matmul(lg[:, 512 * pt:512 * (pt + 1)],
                                             lhsT, rhs, start=True, stop=True)
                        nc.scalar.activation(E[:], lg[:], ACTF.Exp,
                                             bias=EXP_BIAS, scale=1.0 / S_K)
                        vt = vtA if ni == 0 else vtB
                        nc.tensor.matmul(
                            o_ps[par],
                            _fap(vt[:], 0, [[128, 2], [1, 128]]),
                            _fap(E[:], 0, [[512, 2], [1, 512]]),
                            start=(ni == 0), stop=(ni == 1), perf_mode=DR)
                        for lt in range(4):
                            c = 8 * par + 4 * ni + lt
                            for pt in range(2):
                                nc.tensor.matmul(
                                    dallT[:, c:c + 1],
                                    E[:, 512 * pt + 128 * lt:
                                        512 * pt + 128 * (lt + 1)],
                                    ones8[:, 0:1],
                                    start=(pt == 0), stop=(pt == 1))
                        filler()
                # denominator chain: reciprocal, PE transpose (same bank),
                # DRAM bounce, partition-bcast, normalize STT
                rec_sb = recp.tile([128, 16], F32, tag="rec", name="rec")
                nc.vector.reciprocal_approx_fast(rec_sb[:], dallT[:, 0:16])
                nc.tensor.transpose(dallT[0:16, 16:144], rec_sb[:],
                                    ident[:, :])
                recT = recp.tile([16, 128], BF16, tag="recT", name="recT")
                nc.vector.tensor_copy(recT[:], dallT[0:16, 16:144])
                dscD = drp.tile([16, 128], BF16, tag="dscD", name="dscD")
                nc.gpsimd.dma_start(out=dscD[:], in_=recT[:])
                filler()
                for par in range(2):
                    rbc = rbcp.tile([128, 512], BF16, tag="rbc", name="rbc")
                    bsrc = bass.AP(tensor=dscD.tensor,
                                   offset=dscD[:].offset + 8 * par * 128,
                                   ap=[[4 * 128, 2], [0, 64], [128, 4],
                                       [1, 128]])
                    nc.gpsimd.dma_start(out=rbc[:], in_=bsrc)
                    out_ap = _fap(o_resh[:], 1024 * n2 + par,
                                  [[16, 64], [2, 8]])
                    nc.vector.scalar_tensor_tensor(
                        out=out_ap, in0=o_ps[par],
                        scalar=1.0, in1=rbc[:],
                        op0=ALU.mult, op1=ALU.mult)
                filler()

        def back_mk(b, st):
            o_resh, xts = st["o_resh"], st["xts"]
            ot_all = osb.tile([128, NCH * 1024], F32, tag="outsb",
                              name="outsb")
            thunks = []

            def ch_step(ch):
                ot = ot_all[:, 1024 * ch:1024 * (ch + 1)]
                for shalf in range(2):
                    po = mmp.tile([128, 512], F32, tag="mm", name="po")
                    for kp in range(2):
                        lhsT = _fap(owt_s[:], 1280 * kp + 128 * ch,
                                    [[640, 2], [1, 128]])
                        rhs = _fap(o_resh[:], 2048 * kp + 512 * shalf,
                                   [[1024, 2], [1, 512]])
                        nc.tensor.matmul(po[:], lhsT, rhs, start=(kp == 0),
                                         stop=(kp == 1), perf_mode=DR)
                    sl = slice(512 * shalf, 512 * (shalf + 1))
                    nc.vector.scalar_tensor_tensor(
                        out=ot[:, sl], in0=po[:], scalar=LS_IMM,
                        in1=xts[ch][:, sl], op0=ALU.mult, op1=ALU.add)
                nc.sync.dma_start(
                    out=bass.AP(tensor=out4, offset=b * C * S + 131072 * ch,
                                ap=[[1024, 128], [1, 1024]]),
                    in_=ot[:, :])

            for ch in range(NCH):
                thunks.append(lambda ch=ch: ch_step(ch))
            return thunks

        xalls = {0: load_x(0)}
        sts = {}
        st0, th0 = front_mk(0, xalls[0])
        sts[0] = st0
        for t in th0:
            t()
        if nbatch > 1:
            xalls[1] = load_x(1)
        for b in range(nbatch):
            fill = deque()
            if b - 1 >= 0:
                fill.extend(back_mk(b - 1, sts.pop(b - 1)))
            if b + 1 < nbatch:
                if b + 2 < nbatch:
                    fill.append(lambda b=b: xalls.__setitem__(
                        b + 2, load_x(b + 2)))
                st_next, th_next = front_mk(b + 1, xalls[b + 1])
                sts[b + 1] = st_next
                fill.extend(th_next)
            attn(sts[b], fill)
            while fill:
                fill.popleft()()
        for t in back_mk(nbatch - 1, sts.pop(nbatch - 1)):
            t()

    nc.finalize()
    return nc


def _prep_weights(inputs):
    f8 = mybir.dt.np(F8)
    g = {k: np.asarray(v, dtype=np.float32) for k, v in inputs.items()}

    def bnfold(p):
        sc = g[f"{p}_bn_gamma"] / np.sqrt(g[f"{p}_bn_var"] + EPS)
        sh = g[f"{p}_bn_beta"] - g[f"{p}_bn_mean"] * sc
        return sc, sh

    sc_in, sh_in = bnfold("in")
    sc_k, sh_k = bnfold("k")
    sc_v, sh_v = bnfold("v")

    # qw8 [c%128, (chunk 6, oc 512)] = q_w[oc, c] * SW_Q
    qw8 = np.zeros((128, 6, 512), np.float32)
    qwT = g["q_w"].T * SW_Q                     # [c, oc]
    qw8[:, :NCH, :] = qwT.reshape(NCH, 128, 512).transpose(1, 0, 2)
    # wtap8 [c%128, (tap 9, kpair 3, two 2, col 128)]; col 0:64 k, 64:128 v
    # tap order matches tapgeo: (1,1) first, then the remaining 8
    taps = [(dy, dx) for dy in range(3) for dx in range(3)]
    wtap = np.zeros((128, 9, 3, 2, 128), np.float32)
    kwT = g["k_w"].T * (sc_k[:, None] * (SW_K / 8.0))    # [c, kd]
    vwT = g["v_w"].T * (sc_v[:, None] * SW_V)            # [c, vd]
    dwk = g["k_dw_w"][:, 0]                              # [c, 3, 3]
    dwv = g["v_dw_w"][:, 0]
    for ti, (dy, dx) in enumerate(taps):
        for ch in range(NCH):
            cs = slice(128 * ch, 128 * (ch + 1))
            wtap[:, ti, ch // 2, ch % 2, 0:64] = kwT[cs] * dwk[cs, dy, dx][:, None]
            wtap[:, ti, ch // 2, ch % 2, 64:128] = vwT[cs] * dwv[cs, dy, dx][:, None]
    # owt8 [r, (kpair 2, two 2, c 640)] = out_w[c, nv]*SW_O, nv=(2kp+two)*128+r
    owt = np.zeros((128, 2, 2, 640), np.float32)
    owT = g["out_w"].T * SW_O                    # [nv, c]
    for kp in range(2):
        for two in range(2):
            nv0 = (2 * kp + two) * 128
            owt[:, kp, two, :] = owT[nv0:nv0 + 128]
    kconst = (g["k_w"] @ sh_k)[:, None] * (S_K / 8.0)
    vconst = (g["v_w"] @ sh_v)[:, None]
    bn_sc = sc_in.reshape(NCH, 128).T.copy()
    bn_sh = sh_in.reshape(NCH, 128).T.copy()
    return {
        "qw8": qw8.reshape(128, -1).astype(f8),
        "wtap8": wtap.reshape(128, -1).astype(f8),
        "owt8": owt.reshape(128, -1).astype(f8),
        "kconst": kconst.astype(np.float32),
        "vconst": vconst.astype(np.float32),
        "bn_sc": np.ascontiguousarray(bn_sc),
        "bn_sh": np.ascontiguousarray(bn_sh),
    }


_NC_CACHE = None


def kernel(**inputs):
    global _NC_CACHE
    from concourse.bass_utils import run_bass_kernel_spmd

    ls = np.asarray(inputs["ls_gamma"], dtype=np.float32)
    if _NC_CACHE is None:
        _NC_CACHE = build_nc(float(ls[0]))
    nc = _NC_CACHE

    x = np.ascontiguousarray(np.asarray(inputs["x"], dtype=np.float32))
    base = _prep_weights(inputs)
    in_maps = []
    for c in range(N_CORES):
        m = dict(base)
        m["x"] = x[c * BPC:(c + 1) * BPC]
        in_maps.append(m)

    res = run_bass_kernel_spmd(nc, in_maps, core_ids=list(range(N_CORES)))
    out = np.concatenate([res.results[c]["out"] for c in range(N_CORES)], axis=0)
    return out.astype(np.float32)



# revision 69
# speedup vs baseline: 3.6042x; 1.0277x over previous
"""Trainium2 Bass kernel for nn_MultiHeadSelfAttentionBlock (fp8 DoubleRow).

Data-parallel over batch (B=32 -> 4 per core on 8 cores). Weight prep
(BN fold, transposes, tap-weight products, fp8 quantization with
power-of-2 prescales) is host-side; the kernel gets ready fp8 weights.

Per batch item, software-pipelined (front/back emitted as thunks drained
between attention groups so ACT never starves while the lg ring limits
its run-ahead):
  - front: BN (DVE, per-partition scale/shift) -> xn8 fp8 [128, 6*1024];
    row-phase-separated conv planes xph via 10 SBUF->SBUF DMAs; q proj
    (24 DR matmuls + DVE PSUM->SBUF casts into the torch .view-bug qbuf
    layout; ACT carries nothing but the exps so its stream never
    dilutes); k/v dwconv+BN+proj as 27 DR matmuls over the phase planes
    (stride-2 flat taps, 271-wide with ignored junk cols); kfdup row
    duplication via SBUF->SBUF DMA; v transposed via PE into two
    zero-padded DR lhsT variants (vtA/vtB).
  - attention (per head-pair n2, groups (ni, par)): logits DR-input
    matmuls [p 128, l 512] -> 2-bank lg tile; one exp per group on ACT
    (scale 1/S_K) -> E fp8 [128, (pt, l)]; o-matmul = one DR matmul per
    group accumulating ni=0/1 into a pair-packed [128, 512] psum tile;
    softmax denominators land partition-packed via 8 tiny [l' 128, 1]
    matmuls per group (lhsT = E l-slices, rhs = ones col); per n2: one
    reciprocal_approx_fast [128, 16], PE transpose into spare columns of
    the same psum bank, bf16 DRAM bounce + partition-broadcast DMA, and
    one normalize STT [128, 512] per par scatter-writing o_resh fp8.
  - back: out proj (20 DR matmuls) + residual STT out = po*(ls/64) + x
    on DVE, per-chunk output DMA on the SP queue (loads prefetched two
    batches ahead; queue placement chosen empirically since each DMA
    queue serializes its transfers).
"""

from contextlib import ExitStack

import numpy as np

import concourse.bacc as bacc
import concourse.bass as bass
import concourse.tile as tile
from concourse import mybir
from concourse.masks import make_identity

BF16 = mybir.dt.bfloat16

F32 = mybir.dt.float32
F32R = mybir.dt.float32r
F8 = mybir.dt.float8e4
ALU = mybir.AluOpType
ACTF = mybir.ActivationFunctionType
DR = mybir.MatmulPerfMode.DoubleRow

B, C, H, W = 32, 640, 32, 32
NH, KD, VD = 8, 64, 64
S = H * W            # 1024
P = 256              # key/value positions (16x16)
EPS = 1e-3
N_CORES = 8
BPC = B // N_CORES   # 4 batch items per core
NCH = C // 128       # 5 channel chunks

# fp8 prescales (powers of two; descaled at PSUM->SBUF moves)
SW_Q = 64.0          # q_w
SW_K = 1024.0        # k tap weights (incl. the 1/8 logit scale)
SW_V = 256.0         # v tap weights
SW_O = 64.0          # out_w
S_K = 4.0            # kf8 = 4 * (k_true/8)  -> exp scale = 1/4
EXP_BIAS = 0.0       # actual |logit| max ~0.51 -> exp in [0.6, 1.7], safe for e4m3


def _r(ap):
    return ap.bitcast(F32R)


def _fap(base, free_off, dims):
    """AP with base's partition dim and explicit free dims [[step, count],...]."""
    return bass.AP(tensor=base.tensor, offset=base.offset + free_off,
                   ap=[base.ap[0]] + dims)


def build_nc(ls_scalar, nbatch=BPC, dbg=False):
    nc = bacc.Bacc(None, target_bir_lowering=False, debug=False)

    x4 = nc.dram_tensor("x", [nbatch, C, H, W], F32, kind="ExternalInput")
    qw8 = nc.dram_tensor("qw8", [128, 6 * 512], F8, kind="ExternalInput")
    wtap8 = nc.dram_tensor("wtap8", [128, 9 * 3 * 2 * 128], F8,
                           kind="ExternalInput")
    owt8 = nc.dram_tensor("owt8", [128, 2 * 2 * 640], F8, kind="ExternalInput")
    kconst = nc.dram_tensor("kconst", [64, 1], F32, kind="ExternalInput")
    vconst = nc.dram_tensor("vconst", [64, 1], F32, kind="ExternalInput")
    bn_sc = nc.dram_tensor("bn_sc", [128, NCH], F32, kind="ExternalInput")
    bn_sh = nc.dram_tensor("bn_sh", [128, NCH], F32, kind="ExternalInput")
    out4 = nc.dram_tensor("out", [nbatch, C, H, W], F32, kind="ExternalOutput")

    LS_IMM = float(ls_scalar) / SW_O

    with tile.TileContext(nc) as tc, ExitStack() as ctx:
        wp = ctx.enter_context(tc.tile_pool(name="wp", bufs=1))
        # PSUM banks: mmp 1 + lg 2x2 + op 2 + dall(+rec transpose) 1 = 8
        mmp = ctx.enter_context(tc.tile_pool(name="mmp", bufs=1, space="PSUM"))
        lgp = ctx.enter_context(tc.tile_pool(name="lgp", bufs=2, space="PSUM"))
        opp = ctx.enter_context(tc.tile_pool(name="opp", bufs=1, space="PSUM"))
        dalp = ctx.enter_context(tc.tile_pool(name="dalp", bufs=1,
                                              space="PSUM"))
        # SBUF pools
        xin = ctx.enter_context(tc.tile_pool(name="xin", bufs=3))
        xnp = ctx.enter_context(tc.tile_pool(name="xnp", bufs=3))
        qbp = ctx.enter_context(tc.tile_pool(name="qbp", bufs=3))
        kvs = ctx.enter_context(tc.tile_pool(name="kvs", bufs=3))
        ep = ctx.enter_context(tc.tile_pool(name="ep", bufs=6))
        recp = ctx.enter_context(tc.tile_pool(name="recp", bufs=3))
        rbcp = ctx.enter_context(tc.tile_pool(name="rbcp", bufs=6))
        orp = ctx.enter_context(tc.tile_pool(name="orp", bufs=3))
        osb = ctx.enter_context(tc.tile_pool(name="osb", bufs=2))
        drp = ctx.enter_context(tc.tile_pool(name="drp", bufs=4, space="DRAM"))

        # ---- persistent SBUF weights (DMA'd once) ----
        ident = wp.tile([128, 128], F32, tag="ident", name="ident")
        make_identity(nc, ident[:])
        qw8_s = wp.tile([128, 6 * 512], F8, tag="qw8", name="qw8")
        nc.scalar.dma_start(out=qw8_s[:], in_=qw8[:, :])
        wtap_s = wp.tile([128, 9 * 3 * 2 * 128], F8, tag="wtap", name="wtap")
        nc.gpsimd.dma_start(out=wtap_s[:], in_=wtap8[:, :])
        owt_s = wp.tile([128, 2 * 2 * 640], F8, tag="owt", name="owt")
        nc.scalar.dma_start(out=owt_s[:], in_=owt8[:, :])
        kc_s = wp.tile([64, 1], F32, tag="kc", name="kc")
        nc.scalar.dma_start(out=kc_s[:], in_=kconst[:, :])
        vc_s = wp.tile([64, 1], F32, tag="vc", name="vc")
        nc.scalar.dma_start(out=vc_s[:], in_=vconst[:, :])
        bnsc_s = wp.tile([128, NCH], F32, tag="bnsc", name="bnsc")
        nc.gpsimd.dma_start(out=bnsc_s[:], in_=bn_sc[:, :])
        bnsh_s = wp.tile([128, NCH], F32, tag="bnsh", name="bnsh")
        nc.gpsimd.dma_start(out=bnsh_s[:], in_=bn_sh[:, :])

        # persistent row-phase-separated conv buffer, setup-zeroed.
        # Per chunk: 2 planes [17 rows x 34 cols] (pr = h parity; row r2 holds
        # padded image row 2*r2+pr, data cols 1..32). 6 chunks (chunk 5 zero)
        # so the 45 conv planes pad to 27 DR pairs (wtap8's kpair=2,two=1
        # slot is zero host-side to match). Tap (dy,dx) = one stride-2 flat
        # run of 271 elements per plane (17 junk cols/row at j=16, ignored).
        PLN = 17 * 34        # 578, one phase plane
        CHG = 2 * PLN        # 1156, one chunk (both phases)
        xph = wp.tile([128, 6 * CHG], F8, tag="xph", name="xph")
        ones8 = wp.tile([128, 1], F8, tag="ones8", name="ones8")
        nc.vector.memset(ones8[:], 1.0)
        nc.gpsimd.memset(xph[:, 0:3 * CHG], 0.0)
        nc.gpsimd.memset(xph[:, 3 * CHG:6 * CHG], 0.0)
        taps = [(dy, dx) for dy in range(3) for dx in range(3)]


        # ============ software-pipelined batch loop ============
        # front/back phases are emitted as small thunks drained between
        # attention groups, so no engine sees a multi-us block of foreign
        # work while the lg ring (depth 2) caps ACT run-ahead.
        from collections import deque

        def load_x(b, split=False):
            xall = xin.tile([128, NCH * 1024], F32, tag="xall", name="xall")
            if split:
                # cold-start: per-chunk DMAs on alternating idle queues so
                # BN(ch) starts as soon as its chunk lands
                for ch in range(NCH):
                    eng = nc.sync if ch % 2 == 0 else nc.scalar
                    eng.dma_start(
                        out=xall[:, 1024 * ch:1024 * (ch + 1)],
                        in_=bass.AP(tensor=x4,
                                    offset=b * C * S + ch * 128 * 1024,
                                    ap=[[1024, 128], [1, 1024]]))
                return xall
            nc.sync.dma_start(
                out=xall[:],
                in_=bass.AP(tensor=x4, offset=b * C * S,
                            ap=[[1024, 128], [128 * 1024, NCH], [1, 1024]]))
            return xall

        def front_mk(b, xall):
            st = {"xall": xall}
            xts = [xall[:, 1024 * ch:1024 * (ch + 1)] for ch in range(NCH)]
            st["xts"] = xts
            xn8 = xnp.tile([128, 6 * 1024], F8, tag="xn8", name="xn8")
            qbuf = qbp.tile([128, 4096], F8, tag="qbuf", name="qbuf")
            st["qbuf"] = qbuf
            kvpt = mmp.tile([128, 512], F32, tag="mm", name="kvp")
            kfdup = kvs.tile([128, 256], F8, tag="kfdup", name="kfdup")
            st["kfdup"] = kfdup
            vf = kvs.tile([64, 256], F32, tag="vf", name="vf")
            vtA = kvs.tile([128, 256], F8, tag="vtA", name="vtA")
            vtB = kvs.tile([128, 256], F8, tag="vtB", name="vtB")
            st["vtA"], st["vtB"] = vtA, vtB
            thunks = []

            def bn_ch(ch):
                nc.vector.tensor_scalar(
                    out=xn8[:, 1024 * ch:1024 * (ch + 1)], in0=xts[ch][:],
                    scalar1=bnsc_s[:, ch:ch + 1], scalar2=bnsh_s[:, ch:ch + 1],
                    op0=ALU.mult, op1=ALU.add)
                for pr in range(2):
                    h0 = 1 - pr
                    r20 = 1 - pr
                    nc.gpsimd.dma_start(
                        out=_fap(xph[:], CHG * ch + PLN * pr + 34 * r20 + 1,
                                 [[34, 16], [1, 32]]),
                        in_=_fap(xn8[:], 1024 * ch + 32 * h0,
                                 [[64, 16], [1, 32]]))

            thunks.append(lambda: nc.gpsimd.memset(
                xn8[:, 5 * 1024:6 * 1024], 0.0))
            for ch in range(NCH):
                thunks.append(lambda ch=ch: bn_ch(ch))

            def q_t(t):
                qp = mmp.tile([128, 512], F32, tag="mm", name="qp")
                for k in range(3):
                    lhsT = _fap(xn8[:], 2048 * k + 128 * t,
                                [[1024, 2], [1, 128]])
                    rhs = _fap(qw8_s[:], 1024 * k, [[512, 2], [1, 512]])
                    nc.tensor.matmul(qp[:], lhsT, rhs, start=(k == 0),
                                     stop=(k == 2), perf_mode=DR)
                dst = _fap(qbuf[:], t, [[8, 512]])
                nc.vector.tensor_scalar_mul(dst, qp[:], 1.0 / SW_Q)

            for t in range(8):
                thunks.append(lambda t=t: q_t(t))

            def conv_part(tis):
                for ti in tis:
                    dy, dx = taps[ti]
                    pr = dy % 2
                    toff = PLN * pr + 34 * (dy // 2) + dx
                    for kp in range(3):
                        lhsT = _fap(wtap_s[:], 768 * ti + 256 * kp,
                                    [[128, 2], [1, 128]])
                        rhs = _fap(xph[:], CHG * 2 * kp + toff,
                                   [[CHG, 2], [2, 271]])
                        nc.tensor.matmul(kvpt[:, 0:271], lhsT, rhs,
                                         start=(ti == 0 and kp == 0),
                                         stop=(ti == 8 and kp == 2),
                                         perf_mode=DR)

            thunks.append(lambda: conv_part([0, 1, 2, 3]))
            thunks.append(lambda: conv_part([4, 5, 6, 7, 8]))

            def kv_post():
                gcols = [[17, 16], [1, 16]]
                nc.vector.tensor_scalar(out=kfdup[0:64, :],
                                        in0=_fap(kvpt[0:64], 0, gcols),
                                        scalar1=S_K / SW_K, scalar2=kc_s[:],
                                        op0=ALU.mult, op1=ALU.add)
                nc.sync.dma_start(out=kfdup[64:128, :], in_=kfdup[0:64, :])
                nc.vector.tensor_scalar(out=vf[:],
                                        in0=_fap(kvpt[64:128], 0, gcols),
                                        scalar1=1.0 / SW_V, scalar2=vc_s[:],
                                        op0=ALU.mult, op1=ALU.add)
                nc.gpsimd.memset(vtA[:, 64:128], 0.0)
                nc.gpsimd.memset(vtA[:, 192:256], 0.0)
                nc.gpsimd.memset(vtB[:, 0:64], 0.0)
                nc.gpsimd.memset(vtB[:, 128:192], 0.0)

            def vt_mk(pt):
                tp = mmp.tile([128, 512], F32, tag="mm", name="tp")
                nc.tensor.transpose(tp[:128, 0:64],
                                    vf[:, 128 * pt:128 * (pt + 1)],
                                    ident[0:64, 0:64])
                nc.vector.tensor_copy(vtA[:, 128 * pt:128 * pt + 64],
                                       tp[:128, 0:64])
                nc.vector.tensor_copy(vtB[:, 128 * pt + 64:128 * pt + 128],
                                      tp[:128, 0:64])

            thunks.append(kv_post)
            thunks.append(lambda: vt_mk(0))
            thunks.append(lambda: vt_mk(1))
            return st, thunks

        def attn(st, fill):
            qbuf, kfdup = st["qbuf"], st["kfdup"]
            vtA, vtB = st["vtA"], st["vtB"]
            st["o_resh"] = orp.tile([128, 4096], F8, tag="oresh",
                                    name="oresh")
            o_resh = st["o_resh"]
            def filler():
                if fill:
                    fill.popleft()()

            for n2 in range(4):
                opt = opp.tile([128, 1024], F32, tag="op", name="op")
                o_ps = {par: opt[:, 512 * par:512 * (par + 1)]
                        for par in range(2)}
                dallT = dalp.tile([128, 160], F32, tag="dallT", name="dallT")
                for ni in range(2):
                    n = 2 * n2 + ni
                    for par in range(2):
                        E = ep.tile([128, 1024], F8, tag=f"E{par}",
                                    name=f"E{par}")
                        lg = lgp.tile([128, 1024], F32, tag="lg", name="lg")
                        for pt in range(2):
                            lhsT = kfdup[64 * par:64 * (par + 1),
                                         128 * pt:128 * (pt + 1)]
                            rhs = qbuf[64 * par:64 * (par + 1),
                                       512 * n:512 * (n + 1)]
                            nc.tensor.matmul(lg[:, 512 * pt:512 * (pt + 1)],
                                             lhsT, rhs, start=True, stop=True)
                        nc.scalar.activation(E[:], lg[:], ACTF.Exp,
                                             bias=EXP_BIAS, scale=1.0 / S_K)
                        vt = vtA if ni == 0 else vtB
                        nc.tensor.matmul(
                            o_ps[par],
                            _fap(vt[:], 0, [[128, 2], [1, 128]]),
                            _fap(E[:], 0, [[512, 2], [1, 512]]),
                            start=(ni == 0), stop=(ni == 1), perf_mode=DR)
                        for lt in range(4):
                            c = 8 * par + 4 * ni + lt
                            for pt in range(2):
                                nc.tensor.matmul(
                                    dallT[:, c:c + 1],
                                    E[:, 512 * pt + 128 * lt:
                                        512 * pt + 128 * (lt + 1)],
                                    ones8[:, 0:1],
                                    start=(pt == 0), stop=(pt == 1))
                        filler()
                # denominator chain: reciprocal, PE transpose (same bank),
                # DRAM bounce, partition-bcast, normalize STT
                rec_sb = recp.tile([128, 16], F32, tag="rec", name="rec")
                nc.vector.reciprocal_approx_fast(rec_sb[:], dallT[:, 0:16])
                nc.tensor.transpose(dallT[0:16, 16:144], rec_sb[:],
                                    ident[:, :])
                recT = recp.tile([16, 128], BF16, tag="recT", name="recT")
                nc.vector.tensor_copy(recT[:], dallT[0:16, 16:144])
                dscD = drp.tile([16, 128], BF16, tag="dscD", name="dscD")
                nc.gpsimd.dma_start(out=dscD[:], in_=recT[:])
                filler()
                for par in range(2):
                    rbc = rbcp.tile([128, 512], BF16, tag="rbc", name="rbc")
                    bsrc = bass.AP(tensor=dscD.tensor,
                                   offset=dscD[:].offset + 8 * par * 128,
                                   ap=[[4 * 128, 2], [0, 64], [128, 4],
                                       [1, 128]])
                    nc.gpsimd.dma_start(out=rbc[:], in_=bsrc)
                    out_ap = _fap(o_resh[:], 1024 * n2 + par,
                                  [[16, 64], [2, 8]])
                    nc.vector.scalar_tensor_tensor(
                        out=out_ap, in0=o_ps[par],
                        scalar=1.0, in1=rbc[:],
                        op0=ALU.mult, op1=ALU.mult)
                filler()

        def back_mk(b, st, tail=False):
            o_resh, xts = st["o_resh"], st["xts"]
            ot_all = osb.tile([128, NCH * 1024], F32, tag="outsb",
                              name="outsb")
            thunks = []

            def ch_step(ch):
                ot = ot_all[:, 1024 * ch:1024 * (ch + 1)]
                for shalf in range(2):
                    if tail:
                        # attention is done; use the idle lg banks so po
                        # double-buffers instead of ping-ponging on mmp
                        po = lgp.tile([128, 512], F32, tag="lg", name="po")
                    else:
                        po = mmp.tile([128, 512], F32, tag="mm", name="po")
                    for kp in range(2):
                        lhsT = _fap(owt_s[:], 1280 * kp + 128 * ch,
                                    [[640, 2], [1, 128]])
                        rhs = _fap(o_resh[:], 2048 * kp + 512 * shalf,
                                   [[1024, 2], [1, 512]])
                        nc.tensor.matmul(po[:], lhsT, rhs, start=(kp == 0),
                                         stop=(kp == 1), perf_mode=DR)
                    sl = slice(512 * shalf, 512 * (shalf + 1))
                    nc.vector.scalar_tensor_tensor(
                        out=ot[:, sl], in0=po[:], scalar=LS_IMM,
                        in1=xts[ch][:, sl], op0=ALU.mult, op1=ALU.add)
                nc.sync.dma_start(
                    out=bass.AP(tensor=out4, offset=b * C * S + 131072 * ch,
                                ap=[[1024, 128], [1, 1024]]),
                    in_=ot[:, :])

            for ch in range(NCH):
                thunks.append(lambda ch=ch: ch_step(ch))
            return thunks

        xalls = {0: load_x(0, split=True)}
        sts = {}
        st0, th0 = front_mk(0, xalls[0])
        sts[0] = st0
        for t in th0:
            t()
        if nbatch > 1:
            xalls[1] = load_x(1)
        for b in range(nbatch):
            fill = deque()
            if b - 1 >= 0:
                fill.extend(back_mk(b - 1, sts.pop(b - 1)))
            if b + 1 < nbatch:
                if b + 2 < nbatch:
                    fill.append(lambda b=b: xalls.__setitem__(
                        b + 2, load_x(b + 2)))
                st_next, th_next = front_mk(b + 1, xalls[b + 1])
                sts[b + 1] = st_next
                fill.extend(th_next)
            attn(sts[b], fill)
            while fill:
                fill.popleft()()
        for t in back_mk(nbatch - 1, sts.pop(nbatch - 1), tail=True):
            t()

    nc.finalize()
    return nc


def _prep_weights(inputs):
    f8 = mybir.dt.np(F8)
    g = {k: np.asarray(v, dtype=np.float32) for k, v in inputs.items()}

    def bnfold(p):
        sc = g[f"{p}_bn_gamma"] / np.sqrt(g[f"{p}_bn_var"] + EPS)
        sh = g[f"{p}_bn_beta"] - g[f"{p}_bn_mean"] * sc
        return sc, sh

    sc_in, sh_in = bnfold("in")
    sc_k, sh_k = bnfold("k")
    sc_v, sh_v = bnfold("v")

    # qw8 [c%128, (chunk 6, oc 512)] = q_w[oc, c] * SW_Q
    qw8 = np.zeros((128, 6, 512), np.float32)
    qwT = g["q_w"].T * SW_Q                     # [c, oc]
    qw8[:, :NCH, :] = qwT.reshape(NCH, 128, 512).transpose(1, 0, 2)
    # wtap8 [c%128, (tap 9, kpair 3, two 2, col 128)]; col 0:64 k, 64:128 v
    # tap order matches tapgeo: (1,1) first, then the remaining 8
    taps = [(dy, dx) for dy in range(3) for dx in range(3)]
    wtap = np.zeros((128, 9, 3, 2, 128), np.float32)
    kwT = g["k_w"].T * (sc_k[:, None] * (SW_K / 8.0))    # [c, kd]
    vwT = g["v_w"].T * (sc_v[:, None] * SW_V)            # [c, vd]
    dwk = g["k_dw_w"][:, 0]                              # [c, 3, 3]
    dwv = g["v_dw_w"][:, 0]
    for ti, (dy, dx) in enumerate(taps):
        for ch in range(NCH):
            cs = slice(128 * ch, 128 * (ch + 1))
            wtap[:, ti, ch // 2, ch % 2, 0:64] = kwT[cs] * dwk[cs, dy, dx][:, None]
            wtap[:, ti, ch // 2, ch % 2, 64:128] = vwT[cs] * dwv[cs, dy, dx][:, None]
    # owt8 [r, (kpair 2, two 2, c 640)] = out_w[c, nv]*SW_O, nv=(2kp+two)*128+r
    owt = np.zeros((128, 2, 2, 640), np.float32)
    owT = g["out_w"].T * SW_O                    # [nv, c]
    for kp in range(2):
        for two in range(2):
            nv0 = (2 * kp + two) * 128
            owt[:, kp, two, :] = owT[nv0:nv0 + 128]
    kconst = (g["k_w"] @ sh_k)[:, None] * (S_K / 8.0)
    vconst = (g["v_w"] @ sh_v)[:, None]
    bn_sc = sc_in.reshape(NCH, 128).T.copy()
    bn_sh = sh_in.reshape(NCH, 128).T.copy()
    return {
        "qw8": qw8.reshape(128, -1).astype(f8),
        "wtap8": wtap.reshape(128, -1).astype(f8),
        "owt8": owt.reshape(128, -1).astype(f8),
        "kconst": kconst.astype(np.float32),
        "vconst": vconst.astype(np.float32),
        "bn_sc": np.ascontiguousarray(bn_sc),
        "bn_sh": np.ascontiguousarray(bn_sh),
    }


_NC_CACHE = None


def kernel(**inputs):
    global _NC_CACHE
    from concourse.bass_utils import run_bass_kernel_spmd

    ls = np.asarray(inputs["ls_gamma"], dtype=np.float32)
    if _NC_CACHE is None:
        _NC_CACHE = build_nc(float(ls[0]))
    nc = _NC_CACHE

    x = np.ascontiguousarray(np.asarray(inputs["x"], dtype=np.float32))
    base = _prep_weights(inputs)
    in_maps = []
    for c in range(N_CORES):
        m = dict(base)
        m["x"] = x[c * BPC:(c + 1) * BPC]
        in_maps.append(m)

    res = run_bass_kernel_spmd(nc, in_maps, core_ids=list(range(N_CORES)))
    out = np.concatenate([res.results[c]["out"] for c in range(N_CORES)], axis=0)
    return out.astype(np.float32)

